# revision 33
# baseline (speedup 1.0000x reference)
"""GCN layer kernel for nn_GcnNet_17695265259748 — full on-device Bass SPMD.

Pipeline per NeuronCore (8 cores, nodes sharded 6250/core):
  1. stream x shard (host-transposed to [6250,128,20] bf16), reduce over L
     on DVE (contiguous), scale by dis/L on Act -> y_local bf16 [6272,128]
  2. AllGather in two blocks: rows [0:SPLIT) mid-phase-1 (AG0), rest after
     phase 1 (AG1) -> y_full [50176,128] bf16 laid out [lo block | hi block]
  3. two-pass edge processing, per dst tile (49 x 128 dsts):
     pass A (lo): dma_gather chunks whose sources are all in the AG0 block
     (starts as soon as AG0 lands), one-hot M on DVE (batched broadcast
     tensor_tensor bf16), PSUM aggT += G.T @ M, park partial in SBUF f32.
     pass B (hi): remaining chunks after AG1, add parked partial.
     Gathers ride the 4 SWDGE queues (gpsimd cpu pairs) with signed idx16
     rebased per pass; calls span tile boundaries (flat 8-chunk calls).
  4. proj: psum_out = aggT.T @ W + outer(s', b);  out = dis * psum_out
Host does edge preprocessing (sort by dst, lo/hi grouping, index tables).
"""

import sys
import numpy as np

for p in ("/opt/trn_rl_repo",):
    if p not in sys.path:
        sys.path.insert(0, p)

N, L, C, F = 50000, 20, 128, 300
NCORES = 8
NPC = N // NCORES            # 6250 nodes per core
NTILES = (NPC + 127) // 128  # 49 dst tiles (last partial: 106)
NPADC = NTILES * 128         # 6272 rows per core chunk in the table
NROWS = NCORES * NPADC       # 50176 table rows
SPLIT = 1664                 # phase-1 row boundary of the AG halves (t==13)
LO_ROWS = NCORES * SPLIT     # 20480
BASE_LO = LO_ROWS // 2   # 10240: lo-pass chunks hold only lo rows
BASE_HI = NROWS // 2     # 25088: hi-pass chunks may hold any row (leftovers)
MAXCALL = 1024               # dma_gather ring limit (rows per call)
NQ = 4                       # SWDGE queues (gpsimd cpu pairs)

_BUILD_CACHE = {}


def _table_row(src):
    """Row in y_full under the two-block AllGather layout."""
    core = src // NPC
    r = src % NPC
    return np.where(
        r < SPLIT,
        core * SPLIT + r,
        LO_ROWS + core * (NPADC - SPLIT) + (r - SPLIT),
    )


def _preprocess(edge_index):
    """Per-core idx/dstloc tables + uniform per-tile lo/hi chunk counts."""
    row = np.ascontiguousarray(edge_index[0]).astype(np.int64)
    col = np.ascontiguousarray(edge_index[1]).astype(np.int64)
    keep = row != col
    loops = np.arange(N, dtype=np.int64)
    srcs = np.concatenate([row[keep], loops])
    dsts = np.concatenate([col[keep], loops])

    deg = np.bincount(row[keep], minlength=N).astype(np.float64) + 1.0
    dis = (deg ** -0.5).astype(np.float32)
    sprime = np.bincount(dsts, weights=dis[srcs].astype(np.float64), minlength=N)
    sprime = sprime.astype(np.float32)

    order = np.argsort(dsts, kind="stable")
    ds = dsts[order]
    sr = srcs[order]
    trow = _table_row(sr)
    is_lo = trow < LO_ROWS

    core_of = ds // NPC
    tloc = (ds - core_of * NPC) // 128
    gt = core_of * NTILES + tloc
    cnt = np.bincount(gt, minlength=NCORES * NTILES).reshape(NCORES, NTILES)
    lo_cnt = np.bincount(
        gt, weights=is_lo.astype(np.float64), minlength=NCORES * NTILES
    ).astype(np.int64).reshape(NCORES, NTILES)
    CH = np.maximum(((cnt + 127) // 128).max(axis=0), 1)
    CHL = (lo_cnt // 128).min(axis=0)
    CHH = CH - CHL
    assert (CHH >= 0).all()
    NLO = int(CHL.sum())
    NLOP = ((NLO + 7) // 8) * 8            # pad lo pass to call boundary
    CHTOT = NLOP + int(CHH.sum())
    LOb = np.concatenate([[0], np.cumsum(CHL)])[:-1]
    HIb = np.concatenate([[0], np.cumsum(CHH)])[:-1]

    per_core = []
    for c in range(NCORES):
        m = core_of == c
        dsc_ = ds[m]
        src_r = trow[m]
        lo_c = is_lo[m]
        dloc = dsc_ - c * NPC
        tl = dloc // 128
        dl = dloc % 128

        idx = np.zeros((128, CHTOT), dtype=np.int32)   # pad -> rebased 0
        dstloc = np.full((128, CHTOT), -1.0, dtype=np.float32)

        # per tile: first CHL[t]*128 lo edges -> lo chunks; the rest keep
        # their original dst-rank order (mixes lo leftovers with hi edges
        # so no chunk ends up all-negative under the hi rebase)
        korder1 = np.argsort(tl * 2 + (~lo_c), kind="stable")
        ks1 = tl[korder1]
        g1 = np.searchsorted(ks1, np.arange(NTILES))
        rank1 = np.arange(len(ks1)) - g1[ks1]
        early = np.zeros(len(tl), dtype=bool)
        early[korder1] = rank1 < CHL[ks1] * 128
        korder = np.argsort(tl * 2 + (~early), kind="stable")
        ks = tl[korder]
        grp_start = np.searchsorted(ks, np.arange(NTILES))
        rank = np.arange(len(ks)) - grp_start[ks]
        in_lo = rank < CHL[ks] * 128
        gid = np.where(
            in_lo,
            LOb[ks] + rank // 128,
            NLOP + HIb[ks] + (rank - CHL[ks] * 128) // 128,
        )
        pos = rank % 128
        reb = np.where(gid < NLOP, src_r[korder] - BASE_LO, src_r[korder] - BASE_HI)
        idx[pos, gid] = reb
        dstloc[pos, gid] = dl[korder].astype(np.float32)

        # ucode drops TRAILING negative idxs per call; ensure last slot of
        # each flat call >= 0 by swapping within that call's last chunk.
        for call in range((CHTOT + 7) // 8):
            clast = min((call + 1) * 8, CHTOT) - 1
            if idx[127, clast] < 0:
                pp = np.nonzero(idx[:, clast] >= 0)[0]
                assert len(pp), "all-negative chunk"
                p2 = pp[0]
                idx[127, clast], idx[p2, clast] = idx[p2, clast], idx[127, clast]
                dstloc[127, clast], dstloc[p2, clast] = (
                    dstloc[p2, clast],
                    dstloc[127, clast],
                )

        flat = idx.T.reshape(-1)
        assert flat.min() >= -32768 and flat.max() < 32768
        idx16 = flat.astype(np.int16).reshape(-1, 16).T.copy()

        per_core.append(
            {
                "idx16": idx16,
                "dstloc": dstloc,
                "dis": dis[c * NPC : (c + 1) * NPC],
                "sprime": sprime[c * NPC : (c + 1) * NPC],
            }
        )
    return per_core, CH, CHL, CHTOT, dis


def _build(structure):
    """Build the SPMD Bass program. structure = (tuple(CH), tuple(CHL))."""
    key = structure
    if key in _BUILD_CACHE:
        return _BUILD_CACHE[key]

    import concourse.bass as bass
    import concourse.bacc as bacc
    import concourse.mybir as mybir
    import concourse.tile as tile

    CH, CHL = (list(v) for v in structure)
    CHH = [a - b for a, b in zip(CH, CHL)]
    NLO = sum(CHL)
    NLOP = ((NLO + 7) // 8) * 8
    CHTOT = NLOP + sum(CHH)
    LOb = np.concatenate([[0], np.cumsum(CHL)])[:-1].astype(int)
    HIb = np.concatenate([[0], np.cumsum(CHH)])[:-1].astype(int)

    nc = bacc.Bacc(
        None, target_bir_lowering=False, debug=False, num_swdge_queues=NQ
    )
    x_in = nc.dram_tensor("x", [NPC, C, L], mybir.dt.bfloat16, kind="ExternalInput")
    dsc_in = nc.dram_tensor("dscale", [128, NTILES], mybir.dt.float32, kind="ExternalInput")
    disv_in = nc.dram_tensor("disv", [128, NTILES], mybir.dt.float32, kind="ExternalInput")
    sp_in = nc.dram_tensor("sp", [1, NPADC], mybir.dt.float32, kind="ExternalInput")
    idx_in = nc.dram_tensor("idx", [16, CHTOT * 8], mybir.dt.int16, kind="ExternalInput")
    ioidx_in = nc.dram_tensor("ioidx", [16, 8], mybir.dt.int16, kind="ExternalInput")
    dl_in = nc.dram_tensor("dstloc", [128, CHTOT, 1], mybir.dt.bfloat16, kind="ExternalInput")
    iota_in = nc.dram_tensor("iota", [128, 8, 128], mybir.dt.bfloat16, kind="ExternalInput")
    w_in = nc.dram_tensor("W", [C, F], mybir.dt.float32, kind="ExternalInput")
    b_in = nc.dram_tensor("b", [1, F], mybir.dt.float32, kind="ExternalInput")
    out = nc.dram_tensor("out", [NPADC, F], mybir.dt.float32, kind="ExternalOutput")

    with tile.TileContext(nc) as tc:
        with (
            tc.tile_pool(name="sb", bufs=2) as sb,
            tc.tile_pool(name="cst", bufs=1) as cst,
            tc.tile_pool(name="sbp", bufs=4) as sbp,
            tc.tile_pool(name="sbg", bufs=14) as sbg,
            tc.tile_pool(name="sbm", bufs=16) as sbm,
            tc.tile_pool(name="sbx", bufs=6) as sbx,
            tc.tile_pool(name="ps", bufs=4, space="PSUM") as ps,
            tc.tile_pool(name="pso", bufs=4, space="PSUM") as pso,
            tc.tile_pool(name="dram", bufs=1, space="DRAM") as dram,
        ):
            y_loc = dram.tile([NPADC, C], mybir.dt.bfloat16)
            y_full = dram.tile([NROWS, C], mybir.dt.bfloat16)
            ccw = dram.tile([NCORES * 128, C], mybir.dt.bfloat16)

            # ---- constants ----
            dsc = cst.tile([128, NTILES], mybir.dt.float32, tag="dsc")
            nc.sync.dma_start(dsc[:], dsc_in[:])
            disv = cst.tile([128, NTILES], mybir.dt.float32, tag="disv")
            nc.sync.dma_start(disv[:], disv_in[:])
            spv = cst.tile([1, NPADC], mybir.dt.float32, tag="spv")
            nc.sync.dma_start(spv[:], sp_in[:])
            spb = cst.tile([1, NPADC], mybir.dt.bfloat16, tag="spb")
            nc.vector.tensor_copy(spb[:], spv[:])
            iot8 = cst.tile([128, 8, 128], mybir.dt.bfloat16, tag="iot8")
            nc.sync.dma_start(iot8[:], iota_in[:])
            w32 = cst.tile([128, F], mybir.dt.float32, tag="w32")
            nc.sync.dma_start(w32[:], w_in[:])
            wb = cst.tile([128, F], mybir.dt.bfloat16, tag="wb")
            nc.vector.tensor_copy(wb[:], w32[:])
            b32 = cst.tile([1, F], mybir.dt.float32, tag="b32")
            nc.sync.dma_start(b32[:], b_in[:])
            bb = cst.tile([1, F], mybir.dt.bfloat16, tag="bb")
            nc.vector.tensor_copy(bb[:], b32[:])
            aggLo = cst.tile([128, NTILES * 128], mybir.dt.float32, tag="aggLo")

            # ---- phase 1: mean over L, scale, write y_local ----
            ioidx = cst.tile([128, 8], mybir.dt.int16, tag="ioidx")
            for k in range(8):
                nc.sync.dma_start(ioidx[16 * k : 16 * (k + 1), :], ioidx_in[:])
            it = cst.tile([128, CHTOT * 8], mybir.dt.int16, tag="it")
            dltb = cst.tile([128, CHTOT, 1], mybir.dt.bfloat16, tag="dltb")
            # ---- phase 3 issue helpers (defined early so phase 1 can
            # pre-build one-hots) ----
            gather_bufs = {}
            onehot_bufs = {}

            def _issue_gather(call):
                c0 = call * 8
                take = min(8, CHTOT - c0)
                src = (
                    y_full[BASE_LO:LO_ROWS, :]
                    if call < NLOP // 8
                    else y_full[BASE_HI:, :]
                )
                gb = sbg.tile([128, 8, C], mybir.dt.bfloat16, tag="g")
                nc.gpsimd.dma_gather(
                    gb[:, :take, :],
                    src,
                    it[:, c0 * 8 : (c0 + take) * 8],
                    take * 128,
                    take * 128,
                    C,
                    queue_num=call % NQ,
                )
                gather_bufs[call] = gb

            def _issue_onehot(call):
                c0 = call * 8
                take = min(8, CHTOT - c0)
                mb = sbm.tile([128, 8, 128], mybir.dt.bfloat16, tag="m")
                nc.vector.tensor_tensor(
                    out=mb[:, :take, :],
                    in0=iot8[:, :take, :],
                    in1=dltb[:, c0 : c0 + take, :].to_broadcast([128, take, 128]),
                    op=mybir.AluOpType.is_equal,
                )
                onehot_bufs[call] = mb

            def _mm_chunk(pt, gid, start, stop):
                call, slot = gid // 8, gid % 8
                if call not in gather_bufs:
                    _issue_gather(call)
                if call not in onehot_bufs:
                    _issue_onehot(call)
                gb = gather_bufs[call]
                mb = onehot_bufs[call]
                nc.tensor.matmul(
                    pt[:], gb[:, slot, :], mb[:, slot, :], start=start, stop=stop
                )

            for t in range(NTILES):
                n0 = t * 128
                n1 = min(NPC, n0 + 128)
                nn = n1 - n0
                xt = sbx.tile([128, C, L], mybir.dt.bfloat16, tag="xt")
                if t < SPLIT // 128:
                    dma_eng = (nc.sync, nc.gpsimd, nc.scalar)[t % 3]
                else:
                    dma_eng = (nc.sync, nc.scalar)[t % 2]
                dma_eng.dma_start(xt[:nn], x_in[n0:n1])
                if 1 <= t <= 8:
                    k = t - 1
                    dma_eng.dma_start(it[16 * k : 16 * (k + 1), :], idx_in[:])
                elif t == 9:
                    dma_eng.dma_start(dltb[:], dl_in[:])
                xs = sbp.tile([128, C], mybir.dt.float32, tag="xs")
                nc.vector.reduce_sum(xs[:nn], xt[:nn], axis=mybir.AxisListType.X)
                yb = sbp.tile([128, C], mybir.dt.bfloat16, tag="yb")
                if nn < 128:
                    nc.scalar.memzero(yb[:])
                nc.scalar.mul(yb[:nn], xs[:nn], dsc[:nn, t : t + 1])
                (nc.sync if t % 2 == 0 else nc.scalar).dma_start(
                    y_loc[n0 : n0 + 128], yb[:]
                )
                if t >= 37:
                    _issue_onehot(t - 37)
                if t == 0:
                    # warm up the collective engine so AG0 runs at steady
                    # latency (first collective pays a large ramp cost)
                    nc.gpsimd.collective_compute(
                        "AllGather",
                        mybir.AluOpType.bypass,
                        replica_groups=[list(range(NCORES))],
                        ins=[y_loc[0:128].opt()],
                        outs=[ccw.opt()],
                    )
                if t == SPLIT // 128:
                    nc.gpsimd.collective_compute(
                        "AllGather",
                        mybir.AluOpType.bypass,
                        replica_groups=[list(range(NCORES))],
                        ins=[y_loc[0:SPLIT].opt()],
                        outs=[y_full[0:LO_ROWS, :].opt()],
                    )

            # ---- phase 2: AllGather (hi block) ----
            nc.gpsimd.collective_compute(
                "AllGather",
                mybir.AluOpType.bypass,
                replica_groups=[list(range(NCORES))],
                ins=[y_loc[SPLIT:NPADC].opt()],
                outs=[y_full[LO_ROWS:, :].opt()],
            )

            # ---- phase 3: two-pass gather + one-hot scatter matmuls ----
            # pass A: lo chunks -> parked partials
            for t in range(NTILES):
                if CHL[t] == 0:
                    continue
                pt = ps.tile([128, 128], mybir.dt.float32)
                for j in range(CHL[t]):
                    _mm_chunk(pt, int(LOb[t]) + j, j == 0, j == CHL[t] - 1)
                nc.scalar.copy(aggLo[:, t * 128 : (t + 1) * 128], pt[:])

            # pass B: hi chunks + parked partial, then projection
            for t in range(NTILES):
                nhi = CHH[t]
                aggb = sb.tile([128, 128], mybir.dt.bfloat16, tag="aggb")
                agg_slice = aggLo[:, t * 128 : (t + 1) * 128]
                if nhi > 0:
                    pt = ps.tile([128, 128], mybir.dt.float32)
                    for j in range(nhi):
                        _mm_chunk(pt, NLOP + int(HIb[t]) + j, j == 0, j == nhi - 1)
                    if CHL[t] > 0:
                        nc.vector.tensor_tensor(
                            out=aggb[:], in0=pt[:], in1=agg_slice,
                            op=mybir.AluOpType.add,
                        )
                    else:
                        nc.scalar.copy(aggb[:], pt[:])
                else:
                    nc.scalar.copy(aggb[:], agg_slice)
                po = pso.tile([128, F], mybir.dt.float32)
                nc.tensor.matmul(po[:], aggb[:], wb[:], start=True, stop=False)
                nc.tensor.matmul(
                    po[:],
                    spb[:, t * 128 : (t + 1) * 128],
                    bb[:],
                    start=False,
                    stop=True,
                )
                ot = sb.tile([128, F], mybir.dt.float32, tag="ot")
                nc.scalar.mul(ot[:], po[:], disv[:, t : t + 1])
                nc.sync.dma_start(out[t * 128 : (t + 1) * 128], ot[:])

    nc.finalize()
    _BUILD_CACHE[key] = nc
    return nc


def kernel(x, edge_index, W, b):
    import ml_dtypes

    x = np.asarray(x, dtype=np.float32)
    edge_index = np.asarray(edge_index)
    W = np.ascontiguousarray(np.asarray(W, dtype=np.float32))
    b = np.ascontiguousarray(np.asarray(b, dtype=np.float32))

    per_core, CH, CHL, CHTOT, dis = _preprocess(edge_index)
    nc = _build((tuple(int(v) for v in CH), tuple(int(v) for v in CHL)))

    ioidx16 = (
        np.arange(128, dtype=np.int16).reshape(8, 16).T.copy()
    )
    iota = np.broadcast_to(
        np.arange(128, dtype=np.float32), (128, 8, 128)
    ).astype(ml_dtypes.bfloat16)
    in_maps = []
    for c in range(NCORES):
        pc = per_core[c]
        dpad = np.zeros(NPADC, dtype=np.float32)
        dpad[:NPC] = pc["dis"]
        spad = np.zeros(NPADC, dtype=np.float32)
        spad[:NPC] = pc["sprime"]
        dsc = np.ascontiguousarray((dpad / L).reshape(NTILES, 128).T)
        dsv = np.ascontiguousarray(dpad.reshape(NTILES, 128).T)
        in_maps.append(
            {
                "x": np.ascontiguousarray(
                    x[c * NPC : (c + 1) * NPC].transpose(0, 2, 1)
                ).astype(ml_dtypes.bfloat16),
                "dscale": dsc,
                "disv": dsv,
                "sp": spad.reshape(1, NPADC),
                "idx": pc["idx16"],
                "ioidx": ioidx16,
                "dstloc": pc["dstloc"].astype(ml_dtypes.bfloat16)[:, :, None],
                "iota": iota,
                "W": W,
                "b": b.reshape(1, F),
            }
        )

    from concourse.bass_utils import run_bass_kernel_spmd

    res = run_bass_kernel_spmd(nc, in_maps, core_ids=list(range(NCORES)))
    out = np.empty((N, F), dtype=np.float32)
    for c in range(NCORES):
        out[c * NPC : (c + 1) * NPC] = res.results[c]["out"][:NPC]
    return out


# revision 34
# speedup vs baseline: 1.0293x; 1.0293x over previous
"""GCN layer kernel for nn_GcnNet_17695265259748 — full on-device Bass SPMD.

Pipeline per NeuronCore (8 cores, nodes sharded 6250/core):
  1. stream x shard (host-transposed to [6250,128,20] bf16), reduce over L
     on DVE (contiguous), scale by dis/L on Act -> y_local bf16 [6272,128]
  2. AllGather in two blocks: rows [0:SPLIT) mid-phase-1 (AG0), rest after
     phase 1 (AG1) -> y_full [50176,128] bf16 laid out [lo block | hi block]
  3. two-pass edge processing, per dst tile (49 x 128 dsts):
     pass A (lo): dma_gather chunks whose sources are all in the AG0 block
     (starts as soon as AG0 lands), one-hot M on DVE (batched broadcast
     tensor_tensor bf16), PSUM aggT += G.T @ M, park partial in SBUF f32.
     pass B (hi): remaining chunks after AG1, add parked partial.
     Gathers ride the 4 SWDGE queues (gpsimd cpu pairs) with signed idx16
     rebased per pass; calls span tile boundaries (flat 8-chunk calls).
  4. proj: psum_out = aggT.T @ W + outer(s', b);  out = dis * psum_out
Host does edge preprocessing (sort by dst, lo/hi grouping, index tables).
"""

import sys
import numpy as np

for p in ("/opt/trn_rl_repo",):
    if p not in sys.path:
        sys.path.insert(0, p)

N, L, C, F = 50000, 20, 128, 300
NCORES = 8
NPC = N // NCORES            # 6250 nodes per core
NTILES = (NPC + 127) // 128  # 49 dst tiles (last partial: 106)
NPADC = NTILES * 128         # 6272 rows per core chunk in the table
NROWS = NCORES * NPADC       # 50176 table rows
SPLIT = 1664                 # phase-1 row boundary of the AG halves (t==13)
LO_ROWS = NCORES * SPLIT     # 20480
BASE_LO = LO_ROWS // 2   # 10240: lo-pass chunks hold only lo rows
BASE_HI = NROWS // 2     # 25088: hi-pass chunks may hold any row (leftovers)
MAXCALL = 1024               # dma_gather ring limit (rows per call)
NQ = 4                       # SWDGE queues (gpsimd cpu pairs)

_BUILD_CACHE = {}


def _table_row(src):
    """Row in y_full under the two-block AllGather layout."""
    core = src // NPC
    r = src % NPC
    return np.where(
        r < SPLIT,
        core * SPLIT + r,
        LO_ROWS + core * (NPADC - SPLIT) + (r - SPLIT),
    )


def _preprocess(edge_index):
    """Per-core idx/dstloc tables + uniform per-tile lo/hi chunk counts."""
    row = np.ascontiguousarray(edge_index[0]).astype(np.int64)
    col = np.ascontiguousarray(edge_index[1]).astype(np.int64)
    keep = row != col
    loops = np.arange(N, dtype=np.int64)
    srcs = np.concatenate([row[keep], loops])
    dsts = np.concatenate([col[keep], loops])

    deg = np.bincount(row[keep], minlength=N).astype(np.float64) + 1.0
    dis = (deg ** -0.5).astype(np.float32)
    sprime = np.bincount(dsts, weights=dis[srcs].astype(np.float64), minlength=N)
    sprime = sprime.astype(np.float32)

    order = np.argsort(dsts, kind="stable")
    ds = dsts[order]
    sr = srcs[order]
    trow = _table_row(sr)
    is_lo = trow < LO_ROWS

    core_of = ds // NPC
    tloc = (ds - core_of * NPC) // 128
    gt = core_of * NTILES + tloc
    cnt = np.bincount(gt, minlength=NCORES * NTILES).reshape(NCORES, NTILES)
    lo_cnt = np.bincount(
        gt, weights=is_lo.astype(np.float64), minlength=NCORES * NTILES
    ).astype(np.int64).reshape(NCORES, NTILES)
    CH = np.maximum(((cnt + 127) // 128).max(axis=0), 1)
    CHL = (lo_cnt // 128).min(axis=0)
    CHH = CH - CHL
    assert (CHH >= 0).all()
    NLO = int(CHL.sum())
    NLOP = ((NLO + 7) // 8) * 8            # pad lo pass to call boundary
    CHTOT = NLOP + int(CHH.sum())
    LOb = np.concatenate([[0], np.cumsum(CHL)])[:-1]
    HIb = np.concatenate([[0], np.cumsum(CHH)])[:-1]

    per_core = []
    for c in range(NCORES):
        m = core_of == c
        dsc_ = ds[m]
        src_r = trow[m]
        lo_c = is_lo[m]
        dloc = dsc_ - c * NPC
        tl = dloc // 128
        dl = dloc % 128

        idx = np.zeros((128, CHTOT), dtype=np.int32)   # pad -> rebased 0
        dstloc = np.full((128, CHTOT), -1.0, dtype=np.float32)

        # per tile: first CHL[t]*128 lo edges -> lo chunks; the rest keep
        # their original dst-rank order (mixes lo leftovers with hi edges
        # so no chunk ends up all-negative under the hi rebase)
        korder1 = np.argsort(tl * 2 + (~lo_c), kind="stable")
        ks1 = tl[korder1]
        g1 = np.searchsorted(ks1, np.arange(NTILES))
        rank1 = np.arange(len(ks1)) - g1[ks1]
        early = np.zeros(len(tl), dtype=bool)
        early[korder1] = rank1 < CHL[ks1] * 128
        korder = np.argsort(tl * 2 + (~early), kind="stable")
        ks = tl[korder]
        grp_start = np.searchsorted(ks, np.arange(NTILES))
        rank = np.arange(len(ks)) - grp_start[ks]
        in_lo = rank < CHL[ks] * 128
        gid = np.where(
            in_lo,
            LOb[ks] + rank // 128,
            NLOP + HIb[ks] + (rank - CHL[ks] * 128) // 128,
        )
        pos = rank % 128
        reb = np.where(gid < NLOP, src_r[korder] - BASE_LO, src_r[korder] - BASE_HI)
        idx[pos, gid] = reb
        dstloc[pos, gid] = dl[korder].astype(np.float32)

        # ucode drops TRAILING negative idxs per call; ensure last slot of
        # each flat call >= 0 by swapping within that call's last chunk.
        for call in range((CHTOT + 7) // 8):
            clast = min((call + 1) * 8, CHTOT) - 1
            if idx[127, clast] < 0:
                pp = np.nonzero(idx[:, clast] >= 0)[0]
                assert len(pp), "all-negative chunk"
                p2 = pp[0]
                idx[127, clast], idx[p2, clast] = idx[p2, clast], idx[127, clast]
                dstloc[127, clast], dstloc[p2, clast] = (
                    dstloc[p2, clast],
                    dstloc[127, clast],
                )

        flat = idx.T.reshape(-1)
        assert flat.min() >= -32768 and flat.max() < 32768
        idx16 = flat.astype(np.int16).reshape(-1, 16).T.copy()

        per_core.append(
            {
                "idx16": idx16,
                "dstloc": dstloc,
                "dis": dis[c * NPC : (c + 1) * NPC],
                "sprime": sprime[c * NPC : (c + 1) * NPC],
            }
        )
    return per_core, CH, CHL, CHTOT, dis


def _build(structure):
    """Build the SPMD Bass program. structure = (tuple(CH), tuple(CHL))."""
    key = structure
    if key in _BUILD_CACHE:
        return _BUILD_CACHE[key]

    import concourse.bass as bass
    import concourse.bacc as bacc
    import concourse.mybir as mybir
    import concourse.tile as tile

    CH, CHL = (list(v) for v in structure)
    CHH = [a - b for a, b in zip(CH, CHL)]
    NLO = sum(CHL)
    NLOP = ((NLO + 7) // 8) * 8
    CHTOT = NLOP + sum(CHH)
    LOb = np.concatenate([[0], np.cumsum(CHL)])[:-1].astype(int)
    HIb = np.concatenate([[0], np.cumsum(CHH)])[:-1].astype(int)

    nc = bacc.Bacc(
        None, target_bir_lowering=False, debug=False, num_swdge_queues=NQ
    )
    x_in = nc.dram_tensor("x", [NPC, C, L], mybir.dt.bfloat16, kind="ExternalInput")
    dsc_in = nc.dram_tensor("dscale", [128, NTILES], mybir.dt.float32, kind="ExternalInput")
    disv_in = nc.dram_tensor("disv", [128, NTILES], mybir.dt.float32, kind="ExternalInput")
    sp_in = nc.dram_tensor("sp", [1, NPADC], mybir.dt.float32, kind="ExternalInput")
    idx_in = nc.dram_tensor("idx", [16, CHTOT * 8], mybir.dt.int16, kind="ExternalInput")
    ioidx_in = nc.dram_tensor("ioidx", [16, 8], mybir.dt.int16, kind="ExternalInput")
    dl_in = nc.dram_tensor("dstloc", [128, CHTOT, 1], mybir.dt.bfloat16, kind="ExternalInput")
    iota_in = nc.dram_tensor("iota", [128, 8, 128], mybir.dt.bfloat16, kind="ExternalInput")
    w_in = nc.dram_tensor("W", [C, F], mybir.dt.float32, kind="ExternalInput")
    b_in = nc.dram_tensor("b", [1, F], mybir.dt.float32, kind="ExternalInput")
    out = nc.dram_tensor("out", [NPADC, F], mybir.dt.float32, kind="ExternalOutput")

    with tile.TileContext(nc) as tc:
        with (
            tc.tile_pool(name="sb", bufs=2) as sb,
            tc.tile_pool(name="cst", bufs=1) as cst,
            tc.tile_pool(name="sbp", bufs=4) as sbp,
            tc.tile_pool(name="sbg", bufs=14) as sbg,
            tc.tile_pool(name="sbm", bufs=16) as sbm,
            tc.tile_pool(name="sbx", bufs=6) as sbx,
            tc.tile_pool(name="ps", bufs=4, space="PSUM") as ps,
            tc.tile_pool(name="pso", bufs=4, space="PSUM") as pso,
            tc.tile_pool(name="dram", bufs=1, space="DRAM") as dram,
        ):
            y_loc = dram.tile([NPADC, C], mybir.dt.bfloat16)
            y_full = dram.tile([NROWS, C], mybir.dt.bfloat16)

            # ---- constants ----
            dsc = cst.tile([128, NTILES], mybir.dt.float32, tag="dsc")
            nc.sync.dma_start(dsc[:], dsc_in[:])
            disv = cst.tile([128, NTILES], mybir.dt.float32, tag="disv")
            nc.sync.dma_start(disv[:], disv_in[:])
            spv = cst.tile([1, NPADC], mybir.dt.float32, tag="spv")
            nc.sync.dma_start(spv[:], sp_in[:])
            spb = cst.tile([1, NPADC], mybir.dt.bfloat16, tag="spb")
            nc.vector.tensor_copy(spb[:], spv[:])
            iot8 = cst.tile([128, 8, 128], mybir.dt.bfloat16, tag="iot8")
            nc.sync.dma_start(iot8[:], iota_in[:])
            w32 = cst.tile([128, F], mybir.dt.float32, tag="w32")
            nc.sync.dma_start(w32[:], w_in[:])
            wb = cst.tile([128, F], mybir.dt.bfloat16, tag="wb")
            nc.vector.tensor_copy(wb[:], w32[:])
            b32 = cst.tile([1, F], mybir.dt.float32, tag="b32")
            nc.sync.dma_start(b32[:], b_in[:])
            bb = cst.tile([1, F], mybir.dt.bfloat16, tag="bb")
            nc.vector.tensor_copy(bb[:], b32[:])
            aggLo = cst.tile([128, NTILES * 128], mybir.dt.float32, tag="aggLo")

            # ---- phase 1: mean over L, scale, write y_local ----
            ioidx = cst.tile([128, 8], mybir.dt.int16, tag="ioidx")
            for k in range(8):
                nc.sync.dma_start(ioidx[16 * k : 16 * (k + 1), :], ioidx_in[:])
            it = cst.tile([128, CHTOT * 8], mybir.dt.int16, tag="it")
            dltb = cst.tile([128, CHTOT, 1], mybir.dt.bfloat16, tag="dltb")
            # ---- phase 3 issue helpers (defined early so phase 1 can
            # pre-build one-hots) ----
            gather_bufs = {}
            onehot_bufs = {}

            def _issue_gather(call):
                c0 = call * 8
                take = min(8, CHTOT - c0)
                src = (
                    y_full[BASE_LO:LO_ROWS, :]
                    if call < NLOP // 8
                    else y_full[BASE_HI:, :]
                )
                gb = sbg.tile([128, 8, C], mybir.dt.bfloat16, tag="g")
                nc.gpsimd.dma_gather(
                    gb[:, :take, :],
                    src,
                    it[:, c0 * 8 : (c0 + take) * 8],
                    take * 128,
                    take * 128,
                    C,
                    queue_num=call % NQ,
                )
                gather_bufs[call] = gb

            def _issue_onehot(call):
                c0 = call * 8
                take = min(8, CHTOT - c0)
                mb = sbm.tile([128, 8, 128], mybir.dt.bfloat16, tag="m")
                nc.vector.tensor_tensor(
                    out=mb[:, :take, :],
                    in0=iot8[:, :take, :],
                    in1=dltb[:, c0 : c0 + take, :].to_broadcast([128, take, 128]),
                    op=mybir.AluOpType.is_equal,
                )
                onehot_bufs[call] = mb

            def _mm_chunk(pt, gid, start, stop):
                call, slot = gid // 8, gid % 8
                if call not in gather_bufs:
                    _issue_gather(call)
                if call not in onehot_bufs:
                    _issue_onehot(call)
                gb = gather_bufs[call]
                mb = onehot_bufs[call]
                nc.tensor.matmul(
                    pt[:], gb[:, slot, :], mb[:, slot, :], start=start, stop=stop
                )

            for t in range(NTILES):
                n0 = t * 128
                n1 = min(NPC, n0 + 128)
                nn = n1 - n0
                xt = sbx.tile([128, C, L], mybir.dt.bfloat16, tag="xt")
                if t < SPLIT // 128:
                    dma_eng = (nc.sync, nc.gpsimd, nc.scalar)[t % 3]
                else:
                    dma_eng = (nc.sync, nc.scalar)[t % 2]
                dma_eng.dma_start(xt[:nn], x_in[n0:n1])
                if 1 <= t <= 8:
                    k = t - 1
                    dma_eng.dma_start(it[16 * k : 16 * (k + 1), :], idx_in[:])
                elif t == 9:
                    dma_eng.dma_start(dltb[:], dl_in[:])
                xs = sbp.tile([128, C], mybir.dt.float32, tag="xs")
                nc.vector.reduce_sum(xs[:nn], xt[:nn], axis=mybir.AxisListType.X)
                yb = sbp.tile([128, C], mybir.dt.bfloat16, tag="yb")
                if nn < 128:
                    nc.scalar.memzero(yb[:])
                nc.scalar.mul(yb[:nn], xs[:nn], dsc[:nn, t : t + 1])
                (nc.sync if t % 2 == 0 else nc.scalar).dma_start(
                    y_loc[n0 : n0 + 128], yb[:]
                )
                if t >= 37:
                    _issue_onehot(t - 37)
                if t == SPLIT // 128:
                    nc.gpsimd.collective_compute(
                        "AllGather",
                        mybir.AluOpType.bypass,
                        replica_groups=[list(range(NCORES))],
                        ins=[y_loc[0:SPLIT].opt()],
                        outs=[y_full[0:LO_ROWS, :].opt()],
                    )

            # ---- phase 2: AllGather (hi block) ----
            nc.gpsimd.collective_compute(
                "AllGather",
                mybir.AluOpType.bypass,
                replica_groups=[list(range(NCORES))],
                ins=[y_loc[SPLIT:NPADC].opt()],
                outs=[y_full[LO_ROWS:, :].opt()],
            )

            # ---- phase 3: two-pass gather + one-hot scatter matmuls ----
            # pass A: lo chunks -> parked partials
            for t in range(NTILES):
                if CHL[t] == 0:
                    continue
                pt = ps.tile([128, 128], mybir.dt.float32)
                for j in range(CHL[t]):
                    _mm_chunk(pt, int(LOb[t]) + j, j == 0, j == CHL[t] - 1)
                nc.scalar.copy(aggLo[:, t * 128 : (t + 1) * 128], pt[:])

            # pass B: hi chunks + parked partial, then projection
            for t in range(NTILES):
                nhi = CHH[t]
                aggb = sb.tile([128, 128], mybir.dt.bfloat16, tag="aggb")
                agg_slice = aggLo[:, t * 128 : (t + 1) * 128]
                if nhi > 0:
                    pt = ps.tile([128, 128], mybir.dt.float32)
                    for j in range(nhi):
                        _mm_chunk(pt, NLOP + int(HIb[t]) + j, j == 0, j == nhi - 1)
                    if CHL[t] > 0:
                        nc.vector.tensor_tensor(
                            out=aggb[:], in0=pt[:], in1=agg_slice,
                            op=mybir.AluOpType.add,
                        )
                    else:
                        nc.scalar.copy(aggb[:], pt[:])
                else:
                    nc.scalar.copy(aggb[:], agg_slice)
                po = pso.tile([128, F], mybir.dt.float32)
                nc.tensor.matmul(po[:], aggb[:], wb[:], start=True, stop=False)
                nc.tensor.matmul(
                    po[:],
                    spb[:, t * 128 : (t + 1) * 128],
                    bb[:],
                    start=False,
                    stop=True,
                )
                ot = sb.tile([128, F], mybir.dt.float32, tag="ot")
                nc.scalar.mul(ot[:], po[:], disv[:, t : t + 1])
                nc.sync.dma_start(out[t * 128 : (t + 1) * 128], ot[:])

    nc.finalize()
    _BUILD_CACHE[key] = nc
    return nc


def kernel(x, edge_index, W, b):
    import ml_dtypes

    x = np.asarray(x, dtype=np.float32)
    edge_index = np.asarray(edge_index)
    W = np.ascontiguousarray(np.asarray(W, dtype=np.float32))
    b = np.ascontiguousarray(np.asarray(b, dtype=np.float32))

    per_core, CH, CHL, CHTOT, dis = _preprocess(edge_index)
    nc = _build((tuple(int(v) for v in CH), tuple(int(v) for v in CHL)))

    ioidx16 = (
        np.arange(128, dtype=np.int16).reshape(8, 16).T.copy()
    )
    iota = np.broadcast_to(
        np.arange(128, dtype=np.float32), (128, 8, 128)
    ).astype(ml_dtypes.bfloat16)
    in_maps = []
    for c in range(NCORES):
        pc = per_core[c]
        dpad = np.zeros(NPADC, dtype=np.float32)
        dpad[:NPC] = pc["dis"]
        spad = np.zeros(NPADC, dtype=np.float32)
        spad[:NPC] = pc["sprime"]
        dsc = np.ascontiguousarray((dpad / L).reshape(NTILES, 128).T)
        dsv = np.ascontiguousarray(dpad.reshape(NTILES, 128).T)
        in_maps.append(
            {
                "x": np.ascontiguousarray(
                    x[c * NPC : (c + 1) * NPC].transpose(0, 2, 1)
                ).astype(ml_dtypes.bfloat16),
                "dscale": dsc,
                "disv": dsv,
                "sp": spad.reshape(1, NPADC),
                "idx": pc["idx16"],
                "ioidx": ioidx16,
                "dstloc": pc["dstloc"].astype(ml_dtypes.bfloat16)[:, :, None],
                "iota": iota,
                "W": W,
                "b": b.reshape(1, F),
            }
        )

    from concourse.bass_utils import run_bass_kernel_spmd

    res = run_bass_kernel_spmd(nc, in_maps, core_ids=list(range(NCORES)))
    out = np.empty((N, F), dtype=np.float32)
    for c in range(NCORES):
        out[c * NPC : (c + 1) * NPC] = res.results[c]["out"][:NPC]
    return out


# revision 36
# speedup vs baseline: 1.0359x; 1.0064x over previous
"""GCN layer kernel for nn_GcnNet_17695265259748 — full on-device Bass SPMD.

Pipeline per NeuronCore (8 cores, nodes sharded 6250/core):
  1. stream x shard (host-transposed to [6250,128,20] bf16), reduce over L
     on DVE (contiguous), scale by dis/L on Act -> y_local bf16 [6272,128]
  2. AllGather in two blocks: rows [0:SPLIT) mid-phase-1 (AG0), rest after
     phase 1 (AG1) -> y_full [50176,128] bf16 laid out [lo block | hi block]
  3. two-pass edge processing, per dst tile (49 x 128 dsts):
     pass A (lo): dma_gather chunks whose sources are all in the AG0 block
     (starts as soon as AG0 lands), one-hot M on DVE (batched broadcast
     tensor_tensor bf16), PSUM aggT += G.T @ M, park partial in SBUF f32.
     pass B (hi): remaining chunks after AG1, add parked partial.
     Gathers ride the 4 SWDGE queues (gpsimd cpu pairs) with signed idx16
     rebased per pass; calls span tile boundaries (flat 8-chunk calls).
  4. proj: psum_out = aggT.T @ W + outer(s', b);  out = dis * psum_out
Host does edge preprocessing (sort by dst, lo/hi grouping, index tables).
"""

import sys
import numpy as np

for p in ("/opt/trn_rl_repo",):
    if p not in sys.path:
        sys.path.insert(0, p)

N, L, C, F = 50000, 20, 128, 300
NCORES = 8
NPC = N // NCORES            # 6250 nodes per core
NTILES = (NPC + 127) // 128  # 49 dst tiles (last partial: 106)
NPADC = NTILES * 128         # 6272 rows per core chunk in the table
NROWS = NCORES * NPADC       # 50176 table rows
SPLIT = 1664                 # phase-1 row boundary of the AG halves (t==13)
LO_ROWS = NCORES * SPLIT     # 20480
BASE_LO = LO_ROWS // 2   # 10240: lo-pass chunks hold only lo rows
BASE_HI = NROWS // 2     # 25088: hi-pass chunks may hold any row (leftovers)
MAXCALL = 1024               # dma_gather ring limit (rows per call)
NQ = 4                       # SWDGE queues (gpsimd cpu pairs)

_BUILD_CACHE = {}


def _table_row(src):
    """Row in y_full under the two-block AllGather layout."""
    core = src // NPC
    r = src % NPC
    return np.where(
        r < SPLIT,
        core * SPLIT + r,
        LO_ROWS + core * (NPADC - SPLIT) + (r - SPLIT),
    )


def _preprocess(edge_index):
    """Per-core idx/dstloc tables + uniform per-tile lo/hi chunk counts."""
    row = np.ascontiguousarray(edge_index[0]).astype(np.int64)
    col = np.ascontiguousarray(edge_index[1]).astype(np.int64)
    keep = row != col
    loops = np.arange(N, dtype=np.int64)
    srcs = np.concatenate([row[keep], loops])
    dsts = np.concatenate([col[keep], loops])

    deg = np.bincount(row[keep], minlength=N).astype(np.float64) + 1.0
    dis = (deg ** -0.5).astype(np.float32)
    sprime = np.bincount(dsts, weights=dis[srcs].astype(np.float64), minlength=N)
    sprime = sprime.astype(np.float32)

    order = np.argsort(dsts, kind="stable")
    ds = dsts[order]
    sr = srcs[order]
    trow = _table_row(sr)
    is_lo = trow < LO_ROWS

    core_of = ds // NPC
    tloc = (ds - core_of * NPC) // 128
    gt = core_of * NTILES + tloc
    cnt = np.bincount(gt, minlength=NCORES * NTILES).reshape(NCORES, NTILES)
    lo_cnt = np.bincount(
        gt, weights=is_lo.astype(np.float64), minlength=NCORES * NTILES
    ).astype(np.int64).reshape(NCORES, NTILES)
    CH = np.maximum(((cnt + 127) // 128).max(axis=0), 1)
    CHL = (lo_cnt // 128).min(axis=0)
    CHH = CH - CHL
    assert (CHH >= 0).all()
    NLO = int(CHL.sum())
    NLOP = ((NLO + 7) // 8) * 8            # pad lo pass to call boundary
    CHTOT = NLOP + int(CHH.sum())
    LOb = np.concatenate([[0], np.cumsum(CHL)])[:-1]
    HIb = np.concatenate([[0], np.cumsum(CHH)])[:-1]

    per_core = []
    for c in range(NCORES):
        m = core_of == c
        dsc_ = ds[m]
        src_r = trow[m]
        lo_c = is_lo[m]
        dloc = dsc_ - c * NPC
        tl = dloc // 128
        dl = dloc % 128

        idx = np.zeros((128, CHTOT), dtype=np.int32)   # pad -> rebased 0
        dstloc = np.full((128, CHTOT), -1.0, dtype=np.float32)

        # per tile: first CHL[t]*128 lo edges -> lo chunks; the rest keep
        # their original dst-rank order (mixes lo leftovers with hi edges
        # so no chunk ends up all-negative under the hi rebase)
        korder1 = np.argsort(tl * 2 + (~lo_c), kind="stable")
        ks1 = tl[korder1]
        g1 = np.searchsorted(ks1, np.arange(NTILES))
        rank1 = np.arange(len(ks1)) - g1[ks1]
        early = np.zeros(len(tl), dtype=bool)
        early[korder1] = rank1 < CHL[ks1] * 128
        korder = np.argsort(tl * 2 + (~early), kind="stable")
        ks = tl[korder]
        grp_start = np.searchsorted(ks, np.arange(NTILES))
        rank = np.arange(len(ks)) - grp_start[ks]
        in_lo = rank < CHL[ks] * 128
        gid = np.where(
            in_lo,
            LOb[ks] + rank // 128,
            NLOP + HIb[ks] + (rank - CHL[ks] * 128) // 128,
        )
        pos = rank % 128
        reb = np.where(gid < NLOP, src_r[korder] - BASE_LO, src_r[korder] - BASE_HI)
        idx[pos, gid] = reb
        dstloc[pos, gid] = dl[korder].astype(np.float32)

        # ucode drops TRAILING negative idxs per call; ensure last slot of
        # each flat call >= 0 by swapping within that call's last chunk.
        for call in range((CHTOT + 7) // 8):
            clast = min((call + 1) * 8, CHTOT) - 1
            if idx[127, clast] < 0:
                pp = np.nonzero(idx[:, clast] >= 0)[0]
                assert len(pp), "all-negative chunk"
                p2 = pp[0]
                idx[127, clast], idx[p2, clast] = idx[p2, clast], idx[127, clast]
                dstloc[127, clast], dstloc[p2, clast] = (
                    dstloc[p2, clast],
                    dstloc[127, clast],
                )

        flat = idx.T.reshape(-1)
        assert flat.min() >= -32768 and flat.max() < 32768
        idx16 = flat.astype(np.int16).reshape(-1, 16).T.copy()

        per_core.append(
            {
                "idx16": idx16,
                "dstloc": dstloc,
                "dis": dis[c * NPC : (c + 1) * NPC],
                "sprime": sprime[c * NPC : (c + 1) * NPC],
            }
        )
    return per_core, CH, CHL, CHTOT, dis


def _build(structure):
    """Build the SPMD Bass program. structure = (tuple(CH), tuple(CHL))."""
    key = structure
    if key in _BUILD_CACHE:
        return _BUILD_CACHE[key]

    import concourse.bass as bass
    import concourse.bacc as bacc
    import concourse.mybir as mybir
    import concourse.tile as tile

    CH, CHL = (list(v) for v in structure)
    CHH = [a - b for a, b in zip(CH, CHL)]
    NLO = sum(CHL)
    NLOP = ((NLO + 7) // 8) * 8
    CHTOT = NLOP + sum(CHH)
    LOb = np.concatenate([[0], np.cumsum(CHL)])[:-1].astype(int)
    HIb = np.concatenate([[0], np.cumsum(CHH)])[:-1].astype(int)

    nc = bacc.Bacc(
        None, target_bir_lowering=False, debug=False, num_swdge_queues=NQ
    )
    x_in = nc.dram_tensor("x", [NPC, C, L], mybir.dt.bfloat16, kind="ExternalInput")
    dsc_in = nc.dram_tensor("dscale", [128, NTILES], mybir.dt.float32, kind="ExternalInput")
    disv_in = nc.dram_tensor("disv", [128, NTILES], mybir.dt.float32, kind="ExternalInput")
    sp_in = nc.dram_tensor("sp", [1, NPADC], mybir.dt.float32, kind="ExternalInput")
    idx_in = nc.dram_tensor("idx", [16, CHTOT * 8], mybir.dt.int16, kind="ExternalInput")
    ioidx_in = nc.dram_tensor("ioidx", [16, 8], mybir.dt.int16, kind="ExternalInput")
    dl_in = nc.dram_tensor("dstloc", [128, CHTOT, 1], mybir.dt.bfloat16, kind="ExternalInput")
    iota_in = nc.dram_tensor("iota", [128, 8, 128], mybir.dt.bfloat16, kind="ExternalInput")
    w_in = nc.dram_tensor("W", [C, F], mybir.dt.float32, kind="ExternalInput")
    b_in = nc.dram_tensor("b", [1, F], mybir.dt.float32, kind="ExternalInput")
    out = nc.dram_tensor("out", [NPADC, F], mybir.dt.float32, kind="ExternalOutput")

    with tile.TileContext(nc) as tc:
        with (
            tc.tile_pool(name="sb", bufs=2) as sb,
            tc.tile_pool(name="cst", bufs=1) as cst,
            tc.tile_pool(name="sbp", bufs=4) as sbp,
            tc.tile_pool(name="sbg", bufs=14) as sbg,
            tc.tile_pool(name="sbm", bufs=16) as sbm,
            tc.tile_pool(name="sbx", bufs=6) as sbx,
            tc.tile_pool(name="ps", bufs=4, space="PSUM") as ps,
            tc.tile_pool(name="pso", bufs=4, space="PSUM") as pso,
            tc.tile_pool(name="dram", bufs=1, space="DRAM") as dram,
        ):
            y_loc = dram.tile([NPADC, C], mybir.dt.bfloat16)
            y_full = dram.tile([NROWS, C], mybir.dt.bfloat16)

            # ---- constants ----
            dsc = cst.tile([128, NTILES], mybir.dt.float32, tag="dsc")
            nc.sync.dma_start(dsc[:], dsc_in[:])
            disv = cst.tile([128, NTILES], mybir.dt.float32, tag="disv")
            nc.sync.dma_start(disv[:], disv_in[:])
            spv = cst.tile([1, NPADC], mybir.dt.float32, tag="spv")
            nc.sync.dma_start(spv[:], sp_in[:])
            spb = cst.tile([1, NPADC], mybir.dt.bfloat16, tag="spb")
            nc.vector.tensor_copy(spb[:], spv[:])
            iot8 = cst.tile([128, 8, 128], mybir.dt.bfloat16, tag="iot8")
            nc.sync.dma_start(iot8[:], iota_in[:])
            w32 = cst.tile([128, F], mybir.dt.float32, tag="w32")
            nc.sync.dma_start(w32[:], w_in[:])
            wb = cst.tile([128, F], mybir.dt.bfloat16, tag="wb")
            nc.vector.tensor_copy(wb[:], w32[:])
            b32 = cst.tile([1, F], mybir.dt.float32, tag="b32")
            nc.sync.dma_start(b32[:], b_in[:])
            bb = cst.tile([1, F], mybir.dt.bfloat16, tag="bb")
            nc.vector.tensor_copy(bb[:], b32[:])
            aggLo = cst.tile([128, NTILES * 128], mybir.dt.float32, tag="aggLo")

            # ---- phase 1: mean over L, scale, write y_local ----
            ioidx = cst.tile([128, 8], mybir.dt.int16, tag="ioidx")
            for k in range(8):
                nc.sync.dma_start(ioidx[16 * k : 16 * (k + 1), :], ioidx_in[:])
            it = cst.tile([128, CHTOT * 8], mybir.dt.int16, tag="it")
            dltb = cst.tile([128, CHTOT, 1], mybir.dt.bfloat16, tag="dltb")
            # ---- phase 3 issue helpers (defined early so phase 1 can
            # pre-build one-hots) ----
            gather_bufs = {}
            onehot_bufs = {}

            def _issue_gather(call):
                c0 = call * 8
                take = min(8, CHTOT - c0)
                src = (
                    y_full[BASE_LO:LO_ROWS, :]
                    if call < NLOP // 8
                    else y_full[BASE_HI:, :]
                )
                gb = sbg.tile([128, 8, C], mybir.dt.bfloat16, tag="g")
                nc.gpsimd.dma_gather(
                    gb[:, :take, :],
                    src,
                    it[:, c0 * 8 : (c0 + take) * 8],
                    take * 128,
                    take * 128,
                    C,
                    queue_num=call % NQ,
                )
                gather_bufs[call] = gb

            def _issue_onehot(call):
                c0 = call * 8
                take = min(8, CHTOT - c0)
                mb = sbm.tile([128, 8, 128], mybir.dt.bfloat16, tag="m")
                nc.vector.tensor_tensor(
                    out=mb[:, :take, :],
                    in0=iot8[:, :take, :],
                    in1=dltb[:, c0 : c0 + take, :].to_broadcast([128, take, 128]),
                    op=mybir.AluOpType.is_equal,
                )
                onehot_bufs[call] = mb

            def _mm_chunk(pt, gid, start, stop):
                call, slot = gid // 8, gid % 8
                if call not in gather_bufs:
                    _issue_gather(call)
                if call not in onehot_bufs:
                    _issue_onehot(call)
                gb = gather_bufs[call]
                mb = onehot_bufs[call]
                nc.tensor.matmul(
                    pt[:], gb[:, slot, :], mb[:, slot, :], start=start, stop=stop
                )

            for t in range(NTILES):
                n0 = t * 128
                n1 = min(NPC, n0 + 128)
                nn = n1 - n0
                xt = sbx.tile([128, C, L], mybir.dt.bfloat16, tag="xt")
                if t < SPLIT // 128:
                    dma_eng = (nc.sync, nc.gpsimd, nc.scalar)[t % 3]
                else:
                    dma_eng = (nc.sync, nc.scalar)[t % 2]
                dma_eng.dma_start(xt[:nn], x_in[n0:n1])
                if 1 <= t <= 8:
                    k = t - 1
                    dma_eng.dma_start(it[16 * k : 16 * (k + 1), :], idx_in[:])
                elif t == 9:
                    dma_eng.dma_start(dltb[:], dl_in[:])
                xs = sbp.tile([128, C], mybir.dt.float32, tag="xs")
                nc.vector.reduce_sum(xs[:nn], xt[:nn], axis=mybir.AxisListType.X)
                yb = sbp.tile([128, C], mybir.dt.bfloat16, tag="yb")
                if nn < 128:
                    nc.scalar.memzero(yb[:])
                nc.scalar.mul(yb[:nn], xs[:nn], dsc[:nn, t : t + 1])
                (nc.sync if t % 2 == 0 else nc.scalar).dma_start(
                    y_loc[n0 : n0 + 128], yb[:]
                )
                if t >= 37:
                    _issue_onehot(t - 37)
                if t == SPLIT // 128:
                    nc.gpsimd.collective_compute(
                        "AllGather",
                        mybir.AluOpType.bypass,
                        replica_groups=[list(range(NCORES))],
                        ins=[y_loc[0:SPLIT].opt()],
                        outs=[y_full[0:LO_ROWS, :].opt()],
                    )

            # ---- phase 2: AllGather (hi block) ----
            nc.gpsimd.collective_compute(
                "AllGather",
                mybir.AluOpType.bypass,
                replica_groups=[list(range(NCORES))],
                ins=[y_loc[SPLIT:NPADC].opt()],
                outs=[y_full[LO_ROWS:, :].opt()],
            )

            # ---- phase 3: two-pass gather + one-hot scatter matmuls ----
            # pass A: lo chunks -> parked partials
            for t in range(NTILES):
                if CHL[t] == 0:
                    continue
                pt = ps.tile([128, 128], mybir.dt.float32)
                for j in range(CHL[t]):
                    _mm_chunk(pt, int(LOb[t]) + j, j == 0, j == CHL[t] - 1)
                nc.scalar.copy(aggLo[:, t * 128 : (t + 1) * 128], pt[:])

            # pass B: hi chunks + parked partial, then projection
            for t in range(NTILES):
                nhi = CHH[t]
                aggb = sb.tile([128, 128], mybir.dt.bfloat16, tag="aggb")
                agg_slice = aggLo[:, t * 128 : (t + 1) * 128]
                if nhi > 0:
                    pt = ps.tile([128, 128], mybir.dt.float32)
                    for j in range(nhi):
                        _mm_chunk(pt, NLOP + int(HIb[t]) + j, j == 0, j == nhi - 1)
                    if CHL[t] > 0:
                        nc.vector.tensor_tensor(
                            out=aggb[:], in0=pt[:], in1=agg_slice,
                            op=mybir.AluOpType.add,
                        )
                    else:
                        nc.scalar.copy(aggb[:], pt[:])
                else:
                    nc.scalar.copy(aggb[:], agg_slice)
                po = pso.tile([128, F], mybir.dt.float32)
                nc.tensor.matmul(po[:], aggb[:], wb[:], start=True, stop=False)
                nc.tensor.matmul(
                    po[:],
                    spb[:, t * 128 : (t + 1) * 128],
                    bb[:],
                    start=False,
                    stop=True,
                )
                ot = sb.tile([128, F], mybir.dt.float32, tag="ot")
                nc.scalar.mul(ot[:], po[:], disv[:, t : t + 1])
                nc.sync.dma_start(out[t * 128 : (t + 1) * 128], ot[:])

    nc.finalize()
    _BUILD_CACHE[key] = nc
    return nc


def kernel(x, edge_index, W, b):
    import ml_dtypes

    x = np.asarray(x, dtype=np.float32)
    edge_index = np.asarray(edge_index)
    W = np.ascontiguousarray(np.asarray(W, dtype=np.float32))
    b = np.ascontiguousarray(np.asarray(b, dtype=np.float32))

    per_core, CH, CHL, CHTOT, dis = _preprocess(edge_index)
    nc = _build((tuple(int(v) for v in CH), tuple(int(v) for v in CHL)))

    ioidx16 = (
        np.arange(128, dtype=np.int16).reshape(8, 16).T.copy()
    )
    iota = np.broadcast_to(
        np.arange(128, dtype=np.float32), (128, 8, 128)
    ).astype(ml_dtypes.bfloat16)
    in_maps = []
    for c in range(NCORES):
        pc = per_core[c]
        dpad = np.zeros(NPADC, dtype=np.float32)
        dpad[:NPC] = pc["dis"]
        spad = np.zeros(NPADC, dtype=np.float32)
        spad[:NPC] = pc["sprime"]
        dsc = np.ascontiguousarray((dpad / L).reshape(NTILES, 128).T)
        dsv = np.ascontiguousarray(dpad.reshape(NTILES, 128).T)
        in_maps.append(
            {
                "x": np.ascontiguousarray(
                    x[c * NPC : (c + 1) * NPC].transpose(0, 2, 1)
                ).astype(ml_dtypes.bfloat16),
                "dscale": dsc,
                "disv": dsv,
                "sp": spad.reshape(1, NPADC),
                "idx": pc["idx16"],
                "ioidx": ioidx16,
                "dstloc": pc["dstloc"].astype(ml_dtypes.bfloat16)[:, :, None],
                "iota": iota,
                "W": W,
                "b": b.reshape(1, F),
            }
        )

    from concourse.bass_utils import run_bass_kernel_spmd

    res = run_bass_kernel_spmd(nc, in_maps, core_ids=list(range(NCORES)))
    out = np.empty((N, F), dtype=np.float32)
    for c in range(NCORES):
        out[c * NPC : (c + 1) * NPC] = res.results[c]["out"][:NPC]
    return out


# revision 37
# speedup vs baseline: 1.0527x; 1.0162x over previous
"""GCN layer kernel for nn_GcnNet_17695265259748 — full on-device Bass SPMD.

Pipeline per NeuronCore (8 cores, nodes sharded 6250/core):
  1. stream x shard (host-transposed to [6250,128,20] bf16), reduce over L
     on DVE (contiguous), scale by dis/L on Act -> y_local bf16 [6272,128]
  2. AllGather in two blocks: rows [0:SPLIT) mid-phase-1 (AG0), rest after
     phase 1 (AG1) -> y_full [50176,128] bf16 laid out [lo block | hi block]
  3. two-pass edge processing, per dst tile (49 x 128 dsts):
     pass A (lo): dma_gather chunks whose sources are all in the AG0 block
     (starts as soon as AG0 lands), one-hot M on DVE (batched broadcast
     tensor_tensor bf16), PSUM aggT += G.T @ M, park partial in SBUF f32.
     pass B (hi): remaining chunks after AG1, add parked partial.
     Gathers ride the 4 SWDGE queues (gpsimd cpu pairs) with signed idx16
     rebased per pass; calls span tile boundaries (flat 8-chunk calls).
  4. proj: psum_out = aggT.T @ W + outer(s', b);  out = dis * psum_out
Host does edge preprocessing (sort by dst, lo/hi grouping, index tables).
"""

import sys
import numpy as np

for p in ("/opt/trn_rl_repo",):
    if p not in sys.path:
        sys.path.insert(0, p)

N, L, C, F = 50000, 20, 128, 300
NCORES = 8
NPC = N // NCORES            # 6250 nodes per core
NTILES = (NPC + 127) // 128  # 49 dst tiles (last partial: 106)
NPADC = NTILES * 128         # 6272 rows per core chunk in the table
NROWS = NCORES * NPADC       # 50176 table rows
SPLIT = 1664                 # phase-1 row boundary of the AG halves (t==13)
LO_ROWS = NCORES * SPLIT     # 20480
BASE_LO = LO_ROWS // 2   # 10240: lo-pass chunks hold only lo rows
BASE_HI = NROWS // 2     # 25088: hi-pass chunks may hold any row (leftovers)
MAXCALL = 1024               # dma_gather ring limit (rows per call)
NQ = 4                       # SWDGE queues (gpsimd cpu pairs)

_BUILD_CACHE = {}


def _table_row(src):
    """Row in y_full under the two-block AllGather layout."""
    core = src // NPC
    r = src % NPC
    return np.where(
        r < SPLIT,
        core * SPLIT + r,
        LO_ROWS + core * (NPADC - SPLIT) + (r - SPLIT),
    )


def _preprocess(edge_index):
    """Per-core idx/dstloc tables + uniform per-tile lo/hi chunk counts."""
    row = np.ascontiguousarray(edge_index[0]).astype(np.int64)
    col = np.ascontiguousarray(edge_index[1]).astype(np.int64)
    keep = row != col
    loops = np.arange(N, dtype=np.int64)
    srcs = np.concatenate([row[keep], loops])
    dsts = np.concatenate([col[keep], loops])

    deg = np.bincount(row[keep], minlength=N).astype(np.float64) + 1.0
    dis = (deg ** -0.5).astype(np.float32)
    sprime = np.bincount(dsts, weights=dis[srcs].astype(np.float64), minlength=N)
    sprime = sprime.astype(np.float32)

    order = np.argsort(dsts, kind="stable")
    ds = dsts[order]
    sr = srcs[order]
    trow = _table_row(sr)
    is_lo = trow < LO_ROWS

    core_of = ds // NPC
    tloc = (ds - core_of * NPC) // 128
    gt = core_of * NTILES + tloc
    cnt = np.bincount(gt, minlength=NCORES * NTILES).reshape(NCORES, NTILES)
    lo_cnt = np.bincount(
        gt, weights=is_lo.astype(np.float64), minlength=NCORES * NTILES
    ).astype(np.int64).reshape(NCORES, NTILES)
    CH = np.maximum(((cnt + 127) // 128).max(axis=0), 1)
    CHL = (lo_cnt // 128).min(axis=0)
    CHH = CH - CHL
    assert (CHH >= 0).all()
    NLO = int(CHL.sum())
    NLOP = ((NLO + 7) // 8) * 8            # pad lo pass to call boundary
    CHTOT = NLOP + int(CHH.sum())
    LOb = np.concatenate([[0], np.cumsum(CHL)])[:-1]
    # hi groups laid out (and executed) most-chunks-first: the final
    # gather call then leaves a minimal PE tail
    border = sorted(range(NTILES), key=lambda u: (-int(CHH[u]), u))
    HIb = np.zeros(NTILES, dtype=np.int64)
    acc = 0
    for u in border:
        HIb[u] = acc
        acc += int(CHH[u])

    per_core = []
    for c in range(NCORES):
        m = core_of == c
        dsc_ = ds[m]
        src_r = trow[m]
        lo_c = is_lo[m]
        dloc = dsc_ - c * NPC
        tl = dloc // 128
        dl = dloc % 128

        idx = np.zeros((128, CHTOT), dtype=np.int32)   # pad -> rebased 0
        dstloc = np.full((128, CHTOT), -1.0, dtype=np.float32)

        # per tile: first CHL[t]*128 lo edges -> lo chunks; the rest keep
        # their original dst-rank order (mixes lo leftovers with hi edges
        # so no chunk ends up all-negative under the hi rebase)
        korder1 = np.argsort(tl * 2 + (~lo_c), kind="stable")
        ks1 = tl[korder1]
        g1 = np.searchsorted(ks1, np.arange(NTILES))
        rank1 = np.arange(len(ks1)) - g1[ks1]
        early = np.zeros(len(tl), dtype=bool)
        early[korder1] = rank1 < CHL[ks1] * 128
        korder = np.argsort(tl * 2 + (~early), kind="stable")
        ks = tl[korder]
        grp_start = np.searchsorted(ks, np.arange(NTILES))
        rank = np.arange(len(ks)) - grp_start[ks]
        in_lo = rank < CHL[ks] * 128
        gid = np.where(
            in_lo,
            LOb[ks] + rank // 128,
            NLOP + HIb[ks] + (rank - CHL[ks] * 128) // 128,
        )
        pos = rank % 128
        reb = np.where(gid < NLOP, src_r[korder] - BASE_LO, src_r[korder] - BASE_HI)
        idx[pos, gid] = reb
        dstloc[pos, gid] = dl[korder].astype(np.float32)

        # ucode drops TRAILING negative idxs per call; ensure last slot of
        # each flat call >= 0 by swapping within that call's last chunk.
        for call in range((CHTOT + 7) // 8):
            clast = min((call + 1) * 8, CHTOT) - 1
            if idx[127, clast] < 0:
                pp = np.nonzero(idx[:, clast] >= 0)[0]
                assert len(pp), "all-negative chunk"
                p2 = pp[0]
                idx[127, clast], idx[p2, clast] = idx[p2, clast], idx[127, clast]
                dstloc[127, clast], dstloc[p2, clast] = (
                    dstloc[p2, clast],
                    dstloc[127, clast],
                )

        flat = idx.T.reshape(-1)
        assert flat.min() >= -32768 and flat.max() < 32768
        idx16 = flat.astype(np.int16).reshape(-1, 16).T.copy()

        per_core.append(
            {
                "idx16": idx16,
                "dstloc": dstloc,
                "dis": dis[c * NPC : (c + 1) * NPC],
                "sprime": sprime[c * NPC : (c + 1) * NPC],
            }
        )
    return per_core, CH, CHL, CHTOT, dis


def _build(structure):
    """Build the SPMD Bass program. structure = (tuple(CH), tuple(CHL))."""
    key = structure
    if key in _BUILD_CACHE:
        return _BUILD_CACHE[key]

    import concourse.bass as bass
    import concourse.bacc as bacc
    import concourse.mybir as mybir
    import concourse.tile as tile

    CH, CHL = (list(v) for v in structure)
    CHH = [a - b for a, b in zip(CH, CHL)]
    NLO = sum(CHL)
    NLOP = ((NLO + 7) // 8) * 8
    CHTOT = NLOP + sum(CHH)
    LOb = np.concatenate([[0], np.cumsum(CHL)])[:-1].astype(int)
    border = sorted(range(NTILES), key=lambda u: (-int(CHH[u]), u))
    HIb = np.zeros(NTILES, dtype=np.int64)
    acc = 0
    for u in border:
        HIb[u] = acc
        acc += int(CHH[u])

    nc = bacc.Bacc(
        None, target_bir_lowering=False, debug=False, num_swdge_queues=NQ
    )
    x_in = nc.dram_tensor("x", [NPC, C, L], mybir.dt.bfloat16, kind="ExternalInput")
    dsc_in = nc.dram_tensor("dscale", [128, NTILES], mybir.dt.float32, kind="ExternalInput")
    disv_in = nc.dram_tensor("disv", [128, NTILES], mybir.dt.float32, kind="ExternalInput")
    sp_in = nc.dram_tensor("sp", [1, NPADC], mybir.dt.float32, kind="ExternalInput")
    idx_in = nc.dram_tensor("idx", [16, CHTOT * 8], mybir.dt.int16, kind="ExternalInput")
    ioidx_in = nc.dram_tensor("ioidx", [16, 8], mybir.dt.int16, kind="ExternalInput")
    dl_in = nc.dram_tensor("dstloc", [128, CHTOT, 1], mybir.dt.bfloat16, kind="ExternalInput")
    iota_in = nc.dram_tensor("iota", [128, 8, 128], mybir.dt.bfloat16, kind="ExternalInput")
    w_in = nc.dram_tensor("W", [C, F], mybir.dt.float32, kind="ExternalInput")
    b_in = nc.dram_tensor("b", [1, F], mybir.dt.float32, kind="ExternalInput")
    out = nc.dram_tensor("out", [NPADC, F], mybir.dt.float32, kind="ExternalOutput")

    with tile.TileContext(nc) as tc:
        with (
            tc.tile_pool(name="sb", bufs=2) as sb,
            tc.tile_pool(name="cst", bufs=1) as cst,
            tc.tile_pool(name="sbp", bufs=4) as sbp,
            tc.tile_pool(name="sbg", bufs=14) as sbg,
            tc.tile_pool(name="sbm", bufs=16) as sbm,
            tc.tile_pool(name="sbx", bufs=6) as sbx,
            tc.tile_pool(name="ps", bufs=4, space="PSUM") as ps,
            tc.tile_pool(name="pso", bufs=4, space="PSUM") as pso,
            tc.tile_pool(name="dram", bufs=1, space="DRAM") as dram,
        ):
            y_loc = dram.tile([NPADC, C], mybir.dt.bfloat16)
            y_full = dram.tile([NROWS, C], mybir.dt.bfloat16)

            # ---- constants ----
            dsc = cst.tile([128, NTILES], mybir.dt.float32, tag="dsc")
            nc.sync.dma_start(dsc[:], dsc_in[:])
            disv = cst.tile([128, NTILES], mybir.dt.float32, tag="disv")
            nc.sync.dma_start(disv[:], disv_in[:])
            spv = cst.tile([1, NPADC], mybir.dt.float32, tag="spv")
            nc.sync.dma_start(spv[:], sp_in[:])
            spb = cst.tile([1, NPADC], mybir.dt.bfloat16, tag="spb")
            nc.vector.tensor_copy(spb[:], spv[:])
            iot8 = cst.tile([128, 8, 128], mybir.dt.bfloat16, tag="iot8")
            nc.sync.dma_start(iot8[:], iota_in[:])
            w32 = cst.tile([128, F], mybir.dt.float32, tag="w32")
            nc.sync.dma_start(w32[:], w_in[:])
            wb = cst.tile([128, F], mybir.dt.bfloat16, tag="wb")
            nc.vector.tensor_copy(wb[:], w32[:])
            b32 = cst.tile([1, F], mybir.dt.float32, tag="b32")
            nc.sync.dma_start(b32[:], b_in[:])
            bb = cst.tile([1, F], mybir.dt.bfloat16, tag="bb")
            nc.vector.tensor_copy(bb[:], b32[:])
            aggLo = cst.tile([128, NTILES * 128], mybir.dt.float32, tag="aggLo")

            # ---- phase 1: mean over L, scale, write y_local ----
            ioidx = cst.tile([128, 8], mybir.dt.int16, tag="ioidx")
            for k in range(8):
                nc.sync.dma_start(ioidx[16 * k : 16 * (k + 1), :], ioidx_in[:])
            it = cst.tile([128, CHTOT * 8], mybir.dt.int16, tag="it")
            dltb = cst.tile([128, CHTOT, 1], mybir.dt.bfloat16, tag="dltb")
            # ---- phase 3 issue helpers (defined early so phase 1 can
            # pre-build one-hots) ----
            gather_bufs = {}
            onehot_bufs = {}

            def _issue_gather(call):
                c0 = call * 8
                take = min(8, CHTOT - c0)
                src = (
                    y_full[BASE_LO:LO_ROWS, :]
                    if call < NLOP // 8
                    else y_full[BASE_HI:, :]
                )
                gb = sbg.tile([128, 8, C], mybir.dt.bfloat16, tag="g")
                nc.gpsimd.dma_gather(
                    gb[:, :take, :],
                    src,
                    it[:, c0 * 8 : (c0 + take) * 8],
                    take * 128,
                    take * 128,
                    C,
                    queue_num=call % NQ,
                )
                gather_bufs[call] = gb

            def _issue_onehot(call):
                c0 = call * 8
                take = min(8, CHTOT - c0)
                mb = sbm.tile([128, 8, 128], mybir.dt.bfloat16, tag="m")
                nc.vector.tensor_tensor(
                    out=mb[:, :take, :],
                    in0=iot8[:, :take, :],
                    in1=dltb[:, c0 : c0 + take, :].to_broadcast([128, take, 128]),
                    op=mybir.AluOpType.is_equal,
                )
                onehot_bufs[call] = mb

            def _mm_chunk(pt, gid, start, stop):
                call, slot = gid // 8, gid % 8
                if call not in gather_bufs:
                    _issue_gather(call)
                if call not in onehot_bufs:
                    _issue_onehot(call)
                gb = gather_bufs[call]
                mb = onehot_bufs[call]
                nc.tensor.matmul(
                    pt[:], gb[:, slot, :], mb[:, slot, :], start=start, stop=stop
                )

            for t in range(NTILES):
                n0 = t * 128
                n1 = min(NPC, n0 + 128)
                nn = n1 - n0
                xt = sbx.tile([128, C, L], mybir.dt.bfloat16, tag="xt")
                if t < SPLIT // 128:
                    dma_eng = (nc.sync, nc.gpsimd, nc.scalar)[t % 3]
                else:
                    dma_eng = (nc.sync, nc.scalar)[t % 2]
                dma_eng.dma_start(xt[:nn], x_in[n0:n1])
                if 1 <= t <= 8:
                    k = t - 1
                    dma_eng.dma_start(it[16 * k : 16 * (k + 1), :], idx_in[:])
                elif t == 9:
                    dma_eng.dma_start(dltb[:], dl_in[:])
                xs = sbp.tile([128, C], mybir.dt.float32, tag="xs")
                nc.vector.reduce_sum(xs[:nn], xt[:nn], axis=mybir.AxisListType.X)
                yb = sbp.tile([128, C], mybir.dt.bfloat16, tag="yb")
                if nn < 128:
                    nc.scalar.memzero(yb[:])
                nc.scalar.mul(yb[:nn], xs[:nn], dsc[:nn, t : t + 1])
                (nc.sync if t % 2 == 0 else nc.scalar).dma_start(
                    y_loc[n0 : n0 + 128], yb[:]
                )
                if t >= 37:
                    _issue_onehot(t - 37)
                if t == SPLIT // 128:
                    nc.gpsimd.collective_compute(
                        "AllGather",
                        mybir.AluOpType.bypass,
                        replica_groups=[list(range(NCORES))],
                        ins=[y_loc[0:SPLIT].opt()],
                        outs=[y_full[0:LO_ROWS, :].opt()],
                    )

            # ---- phase 2: AllGather (hi block) ----
            nc.gpsimd.collective_compute(
                "AllGather",
                mybir.AluOpType.bypass,
                replica_groups=[list(range(NCORES))],
                ins=[y_loc[SPLIT:NPADC].opt()],
                outs=[y_full[LO_ROWS:, :].opt()],
            )

            # ---- phase 3: two-pass gather + one-hot scatter matmuls ----
            # pass A: lo chunks -> parked partials
            for t in range(NTILES):
                if CHL[t] == 0:
                    continue
                pt = ps.tile([128, 128], mybir.dt.float32)
                for j in range(CHL[t]):
                    _mm_chunk(pt, int(LOb[t]) + j, j == 0, j == CHL[t] - 1)
                nc.scalar.copy(aggLo[:, t * 128 : (t + 1) * 128], pt[:])

            # pass B: hi chunks + parked partial, then projection
            # (border order: most hi chunks first -> minimal post-gather tail)
            for t in border:
                nhi = CHH[t]
                aggb = sb.tile([128, 128], mybir.dt.bfloat16, tag="aggb")
                agg_slice = aggLo[:, t * 128 : (t + 1) * 128]
                if nhi > 0:
                    pt = ps.tile([128, 128], mybir.dt.float32)
                    for j in range(nhi):
                        _mm_chunk(pt, NLOP + int(HIb[t]) + j, j == 0, j == nhi - 1)
                    if CHL[t] > 0:
                        nc.vector.tensor_tensor(
                            out=aggb[:], in0=pt[:], in1=agg_slice,
                            op=mybir.AluOpType.add,
                        )
                    else:
                        nc.scalar.copy(aggb[:], pt[:])
                else:
                    nc.scalar.copy(aggb[:], agg_slice)
                po = pso.tile([128, F], mybir.dt.float32)
                nc.tensor.matmul(po[:], aggb[:], wb[:], start=True, stop=False)
                nc.tensor.matmul(
                    po[:],
                    spb[:, t * 128 : (t + 1) * 128],
                    bb[:],
                    start=False,
                    stop=True,
                )
                ot = sb.tile([128, F], mybir.dt.float32, tag="ot")
                nc.scalar.mul(ot[:], po[:], disv[:, t : t + 1])
                nc.sync.dma_start(out[t * 128 : (t + 1) * 128], ot[:])

    nc.finalize()
    _BUILD_CACHE[key] = nc
    return nc


def kernel(x, edge_index, W, b):
    import ml_dtypes

    x = np.asarray(x, dtype=np.float32)
    edge_index = np.asarray(edge_index)
    W = np.ascontiguousarray(np.asarray(W, dtype=np.float32))
    b = np.ascontiguousarray(np.asarray(b, dtype=np.float32))

    per_core, CH, CHL, CHTOT, dis = _preprocess(edge_index)
    nc = _build((tuple(int(v) for v in CH), tuple(int(v) for v in CHL)))

    ioidx16 = (
        np.arange(128, dtype=np.int16).reshape(8, 16).T.copy()
    )
    iota = np.broadcast_to(
        np.arange(128, dtype=np.float32), (128, 8, 128)
    ).astype(ml_dtypes.bfloat16)
    in_maps = []
    for c in range(NCORES):
        pc = per_core[c]
        dpad = np.zeros(NPADC, dtype=np.float32)
        dpad[:NPC] = pc["dis"]
        spad = np.zeros(NPADC, dtype=np.float32)
        spad[:NPC] = pc["sprime"]
        dsc = np.ascontiguousarray((dpad / L).reshape(NTILES, 128).T)
        dsv = np.ascontiguousarray(dpad.reshape(NTILES, 128).T)
        in_maps.append(
            {
                "x": np.ascontiguousarray(
                    x[c * NPC : (c + 1) * NPC].transpose(0, 2, 1)
                ).astype(ml_dtypes.bfloat16),
                "dscale": dsc,
                "disv": dsv,
                "sp": spad.reshape(1, NPADC),
                "idx": pc["idx16"],
                "ioidx": ioidx16,
                "dstloc": pc["dstloc"].astype(ml_dtypes.bfloat16)[:, :, None],
                "iota": iota,
                "W": W,
                "b": b.reshape(1, F),
            }
        )

    from concourse.bass_utils import run_bass_kernel_spmd

    res = run_bass_kernel_spmd(nc, in_maps, core_ids=list(range(NCORES)))
    out = np.empty((N, F), dtype=np.float32)
    for c in range(NCORES):
        out[c * NPC : (c + 1) * NPC] = res.results[c]["out"][:NPC]
    return out
